# revision 18
# baseline (speedup 1.0000x reference)
"""BatchTopKSAE Trainium2 kernel (feature-sharded over 8 NeuronCores).

  encode : postT[fc,b] = relu(xT.T @ W_encT + b_enc) as a 3-limb GEMM at a
           common 2^14 product scale: fp16 hi*hi pass + ONE fp8-e4m3
           DoubleRow pass computing both cross terms (w16*xl + wl*x16),
           all accumulating into the same PSUM; eviction descales via
           activation(scale=2^-14).  ~1.2e-5 abs activation error - small
           enough that batch-topk membership matches an fp32 encode to a
           handful of boundary swaps.  x tiles stay resident in SBUF (half
           the batch at a time); weight tiles stream through once, host
           pre-tiled for contiguous DMA.  postT streamed to DRAM fp32.
  top-k  : batch-global threshold t = (k*B)-th largest activation.
           Per (feature,256-chunk) top-8 candidates via DVE max8. Stage-1
           sampled 128-probe ladder + AllReduce run concurrently with the
           encode GEMMs; stage-2 exact probe counts are accumulated
           incrementally (counted in 2-fc groups as tiles finish). Tail
           after encode: AllReduce(32 counts) -> window extract ->
           AllGather -> 128-probe exact pass -> exact t.
  decode : f = postT * (postT >= t) cast bf16, masked per 512-row chunk;
           x_hat chunk = f.T @ W_decT (bf16 GEMM); per-core partial sums
           DMA straight to the output in 4 row chunks (decoder weights
           prefetch during the top-k tail); the host sums the 8 partial
           outputs and adds b_dec (the unshard step for feature sharding).
           A device-side ReduceScatter path is kept behind _host_reduce.

Self-contained: hardcodes problem shapes; toolchain from /opt/trn_rl_repo.
"""
import sys

sys.path.insert(0, "/opt/trn_rl_repo")

import functools

import ml_dtypes
import numpy as np

import concourse.bacc as bacc
import concourse.bass_isa as bass_isa
import concourse.mybir as mybir
import concourse.tile as tile
from concourse import bass_utils


F32 = mybir.dt.float32
BF16 = mybir.dt.bfloat16
F16 = mybir.dt.float16
FP8 = mybir.dt.float8e4
DRMODE = mybir.MatmulPerfMode.DoubleRow
ALU = mybir.AluOpType
ACTF = mybir.ActivationFunctionType

N_CORES = 8
BIG = 1.0e30
NP2 = 32          # stage-2 exact probe count
PSCALE = 2.0 ** 14   # common product scale of the 3-limb encode GEMM


def _ladder(n=128, lo=0.25, hi=16.0):
    return np.geomspace(lo, hi, n).astype(np.float32)


def build(B, D, F, K_total, debug_outputs=False, host_reduce=False):
    """Build the SPMD program (same program all cores; data differs)."""
    FC = F // N_CORES
    assert B % 1024 == 0 and D % 128 == 0 and FC % 128 == 0
    NH = 2                         # batch halves (x resident per half)
    HB = B // NH                   # batch cols per half
    FT = FC // 128                 # feature tiles per core
    DT = D // 128                  # contraction tiles
    CCH = 256                      # candidate chunk length
    NCH = HB // CCH                # chunks per (fc, half)
    SPH = FT * NCH * 8             # cand slots per half (per partition)
    SLOTS = NH * SPH               # total cand slots per partition
    # stage-1 sample: all chunk-cells of fc0+fc1, half 0
    SN = 2 * NCH * 8               # sampled slots per partition
    SCALE = SLOTS / SN
    sigma = float(np.sqrt(max(K_total * (SCALE - 1.0), 1.0)))
    margin = 3.0 * sigma + max(200.0, 0.02 * K_total)
    c_hi = (K_total + margin) / SCALE
    c_lo = max((K_total - margin) / SCALE, 0.0)
    NBH = 4                        # decode/RS row chunks
    RB = B // NBH                  # rows per chunk
    BPC = RB // 128                # batch blocks per chunk
    SH = RB // N_CORES             # shard rows per chunk (RS out)
    GW = N_CORES * 1024            # gathered window size
    Kf = float(K_total)

    nc = bacc.Bacc("TRN2", target_bir_lowering=False, debug=False,
                   num_devices=N_CORES)
    # ---- I/O ----
    # encode operands, 3-limb scheme with common product scale 2^14:
    #   post = (x16*128)@(w16*128) + DoubleRow fp8 [(w16*4)(xl*4096)
    #          + (wl*4096)(x16*4)], all accumulating in one PSUM;
    #   eviction descales by 2^-14.  x16/w16 = fp16(x/w).
    x16_d = nc.dram_tensor("x16", [D, B], F16, kind="ExternalInput")
    # packed per half: [D, NH*2*HB]; per-row [2, HB] = (xl*4096, x16*4)
    xp_d = nc.dram_tensor("xp", [D, 2 * B], FP8, kind="ExternalInput")
    # encoder weights pre-tiled on host: [128, FT*DT*128] so the per-fc
    # slice is a contiguous full-rate DMA (strided 256B-line gathers from
    # a [D, FC] layout stall the PE ~3.3us per feature tile)
    weh_d = nc.dram_tensor("weh", [128, FT * DT * 128], F16,
                           kind="ExternalInput")
    # fp8 pairs per d-tile: [128, FT*DT*2*128] = (w16*4, wl*4096)
    wep_d = nc.dram_tensor("wep", [128, FT * DT * 256], FP8,
                           kind="ExternalInput")
    wd_d = nc.dram_tensor("wd", [FC, D], BF16, kind="ExternalInput")
    be_d = nc.dram_tensor("be", [128, FT], F32, kind="ExternalInput")
    pr1_d = nc.dram_tensor("pr1", [128, 1], F32, kind="ExternalInput")
    prrow_d = nc.dram_tensor("prrow", [1, 128], F32, kind="ExternalInput")
    j2_d = nc.dram_tensor("j2", [1, NP2], F32, kind="ExternalInput")
    j128_d = nc.dram_tensor("j128", [128, 1], F32, kind="ExternalInput")
    j16_d = nc.dram_tensor("j16", [1, 16], F32, kind="ExternalInput")
    if host_reduce:
        out_d = nc.dram_tensor("out", [B, D], BF16, kind="ExternalOutput")
    else:
        out_d = nc.dram_tensor("out", [B // N_CORES, D], BF16,
                               kind="ExternalOutput")
    if debug_outputs:
        dbg_t = nc.dram_tensor("dbg_t", [1, 1], F32, kind="ExternalOutput")
        dbg_cnt = nc.dram_tensor("dbg_cnt", [1, NP2], F32,
                                 kind="ExternalOutput")
        dbg_g1 = nc.dram_tensor("dbg_g1", [1, 128], F32,
                                kind="ExternalOutput")
        dbg_fin = nc.dram_tensor("dbg_fin", [1, 8], F32,
                                 kind="ExternalOutput")
        dbg_cand = nc.dram_tensor("dbg_cand", [128, SLOTS], F32,
                                  kind="ExternalOutput")

    rg = [list(range(N_CORES))]

    with tile.TileContext(nc) as tc:
        with tc.tile_pool(name="sb", bufs=1) as sb, \
             tc.tile_pool(name="ps", bufs=1, space="PSUM") as psp, \
             tc.tile_pool(name="dr", bufs=1, space="DRAM") as drp:

            def st(shape, dtype, tag, bufs=1):
                return sb.tile(shape, dtype, tag=tag, bufs=bufs, name=tag)

            # small constants (DMAs emitted after the first weight
            # prefetches - see emit_consts - to keep them off the
            # startup critical path)
            be_sb = st([128, FT], F32, "be")
            pr1 = st([128, 1], F32, "pr1")
            prrow = st([1, 128], F32, "prrow")
            j2 = st([1, NP2], F32, "j2")
            j128 = st([128, 1], F32, "j128")
            j16 = st([1, 16], F32, "j16")

            def emit_consts():
                nc.sync.dma_start(be_sb[:], be_d.ap())
                nc.sync.dma_start(pr1[:], pr1_d.ap())
                nc.sync.dma_start(prrow[:], prrow_d.ap())
                nc.sync.dma_start(j2[:], j2_d.ap())
                nc.sync.dma_start(j128[:], j128_d.ap())
                nc.sync.dma_start(j16[:], j16_d.ap())

            postT_dram = drp.tile([FC, B], F32, tag="postT", name="postT")
            cand = st([128, SLOTS], F32, "cand")

            # big rotating buffer ring: x tiles during encode, decoder
            # weight tiles during decode ([128, HB] bf16 slots)
            XB = 2 * DT            # x tiles per half (hi+lo per d)
            NBIG = XB + 8
            big_tag = "big"

            stage1 = {}
            cnt2h = []

            def emit_stage1():
                # sampled ladder count + AllReduce (overlaps encode)
                samp_row = drp.tile([128, SN], F32, tag="samp", name="samp")
                nc.sync.dma_start(samp_row[:], cand[:, 0:SN])
                samp_bc = st([128, 128 * SN], F32, "mrg")
                nc.sync.dma_start(
                    samp_bc[:],
                    samp_row[:].rearrange("p s -> (p s)").unsqueeze(0)
                    .to_broadcast([128, 128 * SN]))
                SW = 128 * SN
                cnt1 = st([128, 1], F32, "cnt1")
                nchk1 = (SW + 2047) // 2048
                cparts1 = []
                for q in range(nchk1):
                    lo_, hi_ = q * 2048, min((q + 1) * 2048, SW)
                    scr1 = st([128, hi_ - lo_], BF16, "mrgscr")
                    cp = st([128, 1], F32, f"cnt1p{q}")
                    nc.vector.tensor_scalar(out=scr1[:],
                                            in0=samp_bc[:, lo_:hi_],
                                            scalar1=pr1[:], scalar2=0.0,
                                            op0=ALU.is_ge, op1=ALU.add,
                                            accum_out=cp[:])
                    cparts1.append(cp)
                nc.vector.tensor_copy(cnt1[:], cparts1[0][:])
                for cp in cparts1[1:]:
                    nc.vector.tensor_tensor(out=cnt1[:], in0=cnt1[:],
                                            in1=cp[:], op=ALU.add)
                c1io = drp.tile([1, 128], F32, tag="c1i", name="c1i")
                c1oo = drp.tile([1, 128], F32, tag="c1o", name="c1o")
                nc.sync.dma_start(c1io[:].rearrange("a b -> b a"), cnt1[:])
                nc.gpsimd.collective_compute("AllReduce", ALU.add,
                                             ins=[c1io.opt()],
                                             outs=[c1oo.opt()],
                                             replica_groups=rg)
                g1 = st([1, 128], F32, "g1")
                nc.sync.dma_start(g1[:], c1oo[:])
                stage1["g1"] = g1

            def emit_stage2():
                # pick stage-2 probe range from g1, broadcast probes
                g1 = stage1["g1"]
                fhi = st([1, 128], F32, "fhi")
                nc.vector.tensor_scalar(out=fhi[:], in0=g1[:], scalar1=c_hi,
                                        scalar2=None, op0=ALU.is_ge)
                mh = st([1, 128], F32, "mh")
                nc.vector.tensor_tensor(out=mh[:], in0=prrow[:], in1=fhi[:],
                                        op=ALU.mult)
                p_lo = st([1, 1], F32, "p_lo")
                nc.vector.tensor_reduce(out=p_lo[:], in_=mh[:],
                                        axis=mybir.AxisListType.X,
                                        op=ALU.max)
                flo = st([1, 128], F32, "flo")
                nc.vector.tensor_scalar(out=flo[:], in0=g1[:], scalar1=c_lo,
                                        scalar2=None, op0=ALU.is_le)
                ml = st([1, 128], F32, "ml")
                nfl = st([1, 128], F32, "nfl")
                nc.vector.tensor_scalar(out=nfl[:], in0=flo[:],
                                        scalar1=-BIG, scalar2=BIG,
                                        op0=ALU.mult, op1=ALU.add)
                nc.vector.tensor_tensor(out=ml[:], in0=prrow[:], in1=flo[:],
                                        op=ALU.mult)
                nc.vector.tensor_tensor(out=ml[:], in0=ml[:], in1=nfl[:],
                                        op=ALU.add)
                p_hi = st([1, 1], F32, "p_hi")
                nc.vector.tensor_reduce(out=p_hi[:], in_=ml[:],
                                        axis=mybir.AxisListType.X,
                                        op=ALU.min)
                rng = st([1, 1], F32, "rng")
                nc.vector.tensor_tensor(out=rng[:], in0=p_hi[:], in1=p_lo[:],
                                        op=ALU.subtract)
                probes2 = st([1, NP2], F32, "probes2")
                nc.vector.tensor_scalar(out=probes2[:], in0=j2[:],
                                        scalar1=rng[:], scalar2=p_lo[:],
                                        op0=ALU.mult, op1=ALU.add)
                probes2b = st([128, NP2], F32, "probes2b")
                nc.gpsimd.partition_broadcast(probes2b[:], probes2[:])
                stage1["probes2"] = probes2
                stage1["probes2b"] = probes2b

            def emit_count_slice(tag_i, lo_, n_):
                # exact stage-2 counts over cand[:, lo_:lo_+n_]
                probes2b = stage1["probes2b"]
                c2 = st([128, NP2], F32, f"cnt2s{tag_i}_{lo_}")
                for j in range(NP2):
                    scr2 = st([128, n_], BF16, "mrgscr")
                    nc.vector.tensor_scalar(out=scr2[:],
                                            in0=cand[:, lo_:lo_ + n_],
                                            scalar1=probes2b[:, j:j + 1],
                                            scalar2=0.0, op0=ALU.is_ge,
                                            op1=ALU.add,
                                            accum_out=c2[:, j:j + 1])
                cnt2h.append(c2)

            # ============ Phase 1: encode ============
            seq = [(h, fc) for h in range(NH) for fc in range(FT)]
            w_tiles = {}

            def emit_w(idx):
                h_, fc_ = seq[idx]
                w16 = st([128, DT * 128], F16, "ws", bufs=6)
                nc.sync.dma_start(
                    w16[:], weh_d.ap()[:, fc_ * (DT * 128):
                                       (fc_ + 1) * (DT * 128)])
                wp = st([128, DT * 256], FP8, "ws", bufs=6)
                nc.sync.dma_start(
                    wp[:], wep_d.ap()[:, fc_ * (DT * 256):
                                      (fc_ + 1) * (DT * 256)])
                w_tiles[idx] = (w16, wp)

            emit_w(0)
            emit_w(1)
            emit_consts()
            x_t = None
            for idx, (h, fc) in enumerate(seq):
                if fc == 0:
                    if h == 1:
                        emit_stage2()
                        emit_count_slice(0, 0, SPH)
                    x_t = []
                    for d in range(DT):
                        t16 = st([128, HB], F16, big_tag, bufs=NBIG)
                        nc.sync.dma_start(
                            t16[:], x16_d.ap()[d * 128:(d + 1) * 128,
                                               h * HB:(h + 1) * HB])
                        tp = st([128, 2 * HB], FP8, big_tag, bufs=NBIG)
                        nc.sync.dma_start(
                            tp[:], xp_d.ap()[d * 128:(d + 1) * 128,
                                             h * 2 * HB:(h + 1) * 2 * HB])
                        x_t.append((t16, tp))
                if idx + 2 < len(seq):
                    emit_w(idx + 2)
                w16, wp = w_tiles.pop(idx)
                ps = psp.tile([128, HB], F32, tag="ps", bufs=4,
                              name="ps")
                for d in range(DT):
                    wh = w16[:, d * 128:(d + 1) * 128]
                    wdr = wp[:].rearrange("p (t two q) -> p t two q",
                                          t=DT, two=2)[:, d]
                    x16_t, xp_t = x_t[d]
                    xdr = xp_t[:].rearrange("p (two q) -> p two q", two=2)
                    for c in range(0, HB, 512):
                        nc.tensor.matmul(
                            ps[:, c:c + 512], wh, x16_t[:, c:c + 512],
                            start=(d == 0), stop=False)
                        nc.tensor.matmul(
                            ps[:, c:c + 512], wdr, xdr[:, :, c:c + 512],
                            start=False, stop=(d == DT - 1),
                            perf_mode=DRMODE)
                po = st([128, HB], F32, "ev", bufs=3)
                for c in range(0, HB, 512):
                    nc.scalar.activation(po[:, c:c + 512],
                                         ps[:, c:c + 512], ACTF.Relu,
                                         bias=be_sb[:, fc:fc + 1],
                                         scale=1.0 / PSCALE)
                nc.sync.dma_start(
                    postT_dram[fc * 128:(fc + 1) * 128,
                               h * HB:(h + 1) * HB], po[:])
                for ch in range(NCH):
                    base = ((h * FT + fc) * NCH + ch) * 8
                    nc.vector.max(out=cand[:, base:base + 8],
                                  in_=po[:, ch * CCH:(ch + 1) * CCH])
                if h == 0 and fc == 1:
                    emit_stage1()
                if h == 1 and (fc % 2 == 1 or fc == 14):
                    if fc < 14:
                        g = fc // 2
                        emit_count_slice(1, SPH + g * (SPH // 8), SPH // 8)
                    elif fc == 14:
                        emit_count_slice(1, SPH + 14 * (SPH // 16),
                                         SPH // 16)
                        # reduce everything but fc15 while fc15 computes
                        cpre = st([128, NP2], F32, "cnt2pre")
                        nc.vector.tensor_tensor(out=cpre[:],
                                                in0=cnt2h[0][:],
                                                in1=cnt2h[1][:],
                                                op=ALU.add)
                        for c2x in cnt2h[2:]:
                            nc.vector.tensor_tensor(out=cpre[:],
                                                    in0=cpre[:],
                                                    in1=c2x[:], op=ALU.add)
                        par_a = st([128, NP2], F32, "par_a")
                        nc.gpsimd.partition_all_reduce(
                            par_a[:], cpre[:], channels=128,
                            reduce_op=bass_isa.ReduceOp.add)
                        stage1["par_a"] = par_a
                    else:
                        emit_count_slice(1, SPH + 15 * (SPH // 16),
                                         SPH // 16)

            # ============ Phase 2: finish counts + AllReduce ============
            # only fc15's counts remain; everything else was reduced
            # under fc15's compute shadow
            par_b = st([128, NP2], F32, "par_b")
            nc.gpsimd.partition_all_reduce(par_b[:], cnt2h[-1][:],
                                           channels=128,
                                           reduce_op=bass_isa.ReduceOp.add)
            par2 = st([128, NP2], F32, "par2")
            nc.vector.tensor_tensor(out=par2[:], in0=stage1["par_a"][:],
                                    in1=par_b[:], op=ALU.add)
            c2io = drp.tile([1, NP2], F32, tag="c2i", name="c2i")
            c2oo = drp.tile([1, NP2], F32, tag="c2o", name="c2o")
            nc.sync.dma_start(c2io[:], par2[0:1, :])
            nc.gpsimd.collective_compute("AllReduce", ALU.add,
                                         ins=[c2io.opt()],
                                         outs=[c2oo.opt()],
                                         replica_groups=rg)

            # prefetch decoder weights while the collective tail runs
            wd_t = []
            for fc in range(FT):
                w0 = st([128, HB], BF16, big_tag, bufs=NBIG)
                nc.sync.dma_start(
                    w0[:], wd_d.ap()[fc * 128:(fc + 1) * 128, 0:HB])
                w1 = st([128, HB], BF16, big_tag, bufs=NBIG)
                nc.sync.dma_start(
                    w1[:], wd_d.ap()[fc * 128:(fc + 1) * 128, HB:D])
                wd_t.append((w0, w1))

            g2 = st([1, NP2], F32, "g2")
            nc.sync.dma_start(g2[:], c2oo[:])

            # ============ Phase 3: window pick + extract ============
            probes2 = stage1["probes2"]
            f2 = st([1, NP2], F32, "f2")
            nc.vector.tensor_scalar(out=f2[:], in0=g2[:], scalar1=Kf,
                                    scalar2=None, op0=ALU.is_ge)
            w1s = st([1, NP2], F32, "w1s")
            nc.vector.tensor_tensor(out=w1s[:], in0=probes2[:], in1=f2[:],
                                    op=ALU.mult)
            tau_a = st([1, 1], F32, "tau_a")
            nc.vector.tensor_reduce(out=tau_a[:], in_=w1s[:],
                                    axis=mybir.AxisListType.X, op=ALU.max)
            w2s = st([1, NP2], F32, "w2s")
            nb2 = st([1, NP2], F32, "nb2")
            nc.vector.tensor_scalar(out=nb2[:], in0=f2[:], scalar1=-BIG,
                                    scalar2=BIG, op0=ALU.mult, op1=ALU.add)
            nc.vector.tensor_tensor(out=w2s[:], in0=g2[:], in1=f2[:],
                                    op=ALU.mult)
            nc.vector.tensor_tensor(out=w2s[:], in0=w2s[:], in1=nb2[:],
                                    op=ALU.add)
            C_a = st([1, 1], F32, "C_a")
            nc.vector.tensor_reduce(out=C_a[:], in_=w2s[:],
                                    axis=mybir.AxisListType.X, op=ALU.min)
            nf2 = st([1, NP2], F32, "nf2")
            nc.vector.tensor_scalar(out=nf2[:], in0=f2[:], scalar1=-1.0,
                                    scalar2=1.0, op0=ALU.mult, op1=ALU.add)
            w3s = st([1, NP2], F32, "w3s")
            bf2 = st([1, NP2], F32, "bf2")
            nc.vector.tensor_scalar(out=bf2[:], in0=f2[:], scalar1=BIG,
                                    scalar2=None, op0=ALU.mult)
            nc.vector.tensor_tensor(out=w3s[:], in0=probes2[:], in1=nf2[:],
                                    op=ALU.mult)
            nc.vector.tensor_tensor(out=w3s[:], in0=w3s[:], in1=bf2[:],
                                    op=ALU.add)
            tau_b = st([1, 1], F32, "tau_b")
            nc.vector.tensor_reduce(out=tau_b[:], in_=w3s[:],
                                    axis=mybir.AxisListType.X, op=ALU.min)
            tab = st([128, 1], F32, "tab")
            nc.gpsimd.partition_broadcast(tab[:], tau_a[:])
            tbb = st([128, 1], F32, "tbb")
            nc.gpsimd.partition_broadcast(tbb[:], tau_b[:])
            # window members or 0 (in place over cand; cand's last use)
            nc.vector.scalar_tensor_tensor(out=cand[:], in0=cand[:],
                                           scalar=tab[:], in1=cand[:],
                                           op0=ALU.is_ge, op1=ALU.mult)
            nc.vector.scalar_tensor_tensor(out=cand[:], in0=cand[:],
                                           scalar=tbb[:], in1=cand[:],
                                           op0=ALU.is_lt, op1=ALU.mult)
            wm8 = st([128, 8], F32, "wm8")
            nc.vector.max(out=wm8[:], in_=cand[:])

            # ============ Phase 4: AllGather window + exact t ============
            # prefetch the first mask inputs during the collective tail
            pr_pre = []
            for fcp in range(3):
                prp = st([128, RB], F32, "rld", bufs=3)
                nc.sync.dma_start(
                    prp[:], postT_dram[fcp * 128:(fcp + 1) * 128, 0:RB])
                pr_pre.append(prp)

            win_i = drp.tile([128, 8], F32, tag="win_i", name="win_i")
            win_o = drp.tile([1, GW], F32, tag="win_o", name="win_o")
            nc.sync.dma_start(win_i[:], wm8[:])
            nc.gpsimd.collective_compute("AllGather", ALU.bypass,
                                         ins=[win_i.opt()],
                                         outs=[win_o.opt()],
                                         replica_groups=rg)
            gath = st([128, GW], F32, "mrg")
            nc.sync.dma_start(gath[:], win_o[:].to_broadcast([128, GW]))
            rng3 = st([1, 1], F32, "rng3")
            nc.vector.tensor_tensor(out=rng3[:], in0=tau_b[:],
                                    in1=tau_a[:], op=ALU.subtract)
            rng3b = st([128, 1], F32, "rng3b")
            nc.gpsimd.partition_broadcast(rng3b[:], rng3[:])
            probes3 = st([128, 1], F32, "probes3")
            nc.vector.tensor_scalar(out=probes3[:], in0=j128[:],
                                    scalar1=rng3b[:], scalar2=tab[:],
                                    op0=ALU.mult, op1=ALU.add)
            cnt3 = st([128, 1], F32, "cnt3")
            nchk3 = (GW + 2047) // 2048
            cparts3 = []
            for q in range(nchk3):
                lo_, hi_ = q * 2048, min((q + 1) * 2048, GW)
                scr3 = st([128, hi_ - lo_], BF16, "mrgscr")
                cp3 = st([128, 1], F32, f"cnt3p{q}")
                nc.vector.tensor_scalar(out=scr3[:], in0=gath[:, lo_:hi_],
                                        scalar1=probes3[:], scalar2=0.0,
                                        op0=ALU.is_ge, op1=ALU.add,
                                        accum_out=cp3[:])
                cparts3.append(cp3)
            nc.vector.tensor_copy(cnt3[:], cparts3[0][:])
            for cp3 in cparts3[1:]:
                nc.vector.tensor_tensor(out=cnt3[:], in0=cnt3[:],
                                        in1=cp3[:], op=ALU.add)
            wa = st([128, 1], F32, "wa")
            nc.gpsimd.partition_broadcast(wa[:], cnt3[0:1, :])
            cab = st([128, 1], F32, "cab")
            nc.gpsimd.partition_broadcast(cab[:], C_a[:])
            c3g = st([128, 1], F32, "c3g")
            nc.vector.tensor_tensor(out=c3g[:], in0=cnt3[:], in1=wa[:],
                                    op=ALU.subtract)
            nc.vector.tensor_tensor(out=c3g[:], in0=c3g[:], in1=cab[:],
                                    op=ALU.add)
            f3 = st([128, 1], F32, "f3")
            nc.vector.tensor_scalar(out=f3[:], in0=c3g[:], scalar1=Kf,
                                    scalar2=None, op0=ALU.is_ge)
            pf = st([128, 1], F32, "pf")
            nc.vector.tensor_tensor(out=pf[:], in0=probes3[:], in1=f3[:],
                                    op=ALU.mult)
            tlo = st([128, 1], F32, "tlo")
            nc.gpsimd.partition_all_reduce(tlo[:], pf[:], channels=128,
                                           reduce_op=bass_isa.ReduceOp.max)
            nf3 = st([128, 1], F32, "nf3")
            nc.vector.tensor_scalar(out=nf3[:], in0=f3[:], scalar1=-1.0,
                                    scalar2=1.0, op0=ALU.mult, op1=ALU.add)
            cbv = st([128, 1], F32, "cbv")
            nc.vector.tensor_tensor(out=cbv[:], in0=cab[:], in1=wa[:],
                                    op=ALU.subtract)
            # C_hi = C3 at first unflagged probe = max over unflagged C3
            # (C3 monotone decreasing); all-flagged fallback = C_b.
            m1 = st([128, 1], F32, "m1")
            nc.vector.tensor_tensor(out=m1[:], in0=c3g[:], in1=nf3[:],
                                    op=ALU.mult)
            nc.vector.tensor_tensor(out=m1[:], in0=m1[:], in1=cbv[:],
                                    op=ALU.max)
            chi = st([128, 1], F32, "chi")
            nc.gpsimd.partition_all_reduce(chi[:], m1[:], channels=128,
                                           reduce_op=bass_isa.ReduceOp.max)
            p1m = st([128, 1], F32, "p1m")
            nc.vector.tensor_tensor(out=p1m[:], in0=probes3[:], in1=nf3[:],
                                    op=ALU.mult)
            bigf = st([128, 1], F32, "bigf")
            nc.vector.tensor_scalar(out=bigf[:], in0=f3[:], scalar1=BIG,
                                    scalar2=None, op0=ALU.mult)
            nc.vector.tensor_tensor(out=p1m[:], in0=p1m[:], in1=bigf[:],
                                    op=ALU.add)
            nc.vector.tensor_scalar(out=p1m[:], in0=p1m[:], scalar1=-1.0,
                                    scalar2=None, op0=ALU.mult)
            thi_n = st([128, 1], F32, "thi_n")
            nc.gpsimd.partition_all_reduce(thi_n[:], p1m[:], channels=128,
                                           reduce_op=bass_isa.ReduceOp.max)
            thi = st([128, 1], F32, "thi")
            nc.vector.tensor_scalar(out=thi[:], in0=thi_n[:], scalar1=-1.0,
                                    scalar2=None, op0=ALU.mult)
            # bracket members, partition-distributed ([1,8192] single-lane
            # DVE ops cost ~8.5us each; [128,64] + a small bounce is ~40x
            # less vector time)
            wdist = st([128, GW // 128], F32, "wdist")
            nc.sync.dma_start(
                wdist[:],
                win_o[:].rearrange("a (p q) -> (a p) q", p=128))
            nc.vector.scalar_tensor_tensor(out=wdist[:], in0=wdist[:],
                                           scalar=tlo[:], in1=wdist[:],
                                           op0=ALU.is_ge, op1=ALU.mult)
            nc.vector.scalar_tensor_tensor(out=wdist[:], in0=wdist[:],
                                           scalar=thi[:], in1=wdist[:],
                                           op0=ALU.is_lt, op1=ALU.mult)
            wm8b = st([128, 8], F32, "wm8b")
            nc.vector.max(out=wm8b[:], in_=wdist[:])
            zrow_d = drp.tile([1, 1024], F32, tag="zrow", name="zrow")
            nc.sync.dma_start(
                zrow_d[:].rearrange("a (p q) -> (a p) q", p=128), wm8b[:])
            zbc = st([1, 1024], F32, "zbc")
            nc.sync.dma_start(zbc[:], zrow_d[:])
            z = st([1, 16], F32, "z16")
            nc.vector.max(out=z[:, 0:8], in_=zbc[:])
            nc.vector.match_replace(out=zbc[:], in_to_replace=z[:, 0:8],
                                    in_values=zbc[:], imm_value=0.0)
            nc.vector.max(out=z[:, 8:16], in_=zbc[:])
            rm1 = st([1, 1], F32, "rm1")
            nc.vector.tensor_scalar(out=rm1[:], in0=chi[0:1, :],
                                    scalar1=-1.0, scalar2=Kf - 1.0,
                                    op0=ALU.mult, op1=ALU.add)
            fr = st([1, 16], F32, "fr")
            nc.vector.tensor_scalar(out=fr[:], in0=j16[:], scalar1=rm1[:],
                                    scalar2=None, op0=ALU.is_equal)
            zt = st([1, 16], F32, "zt")
            nc.vector.tensor_tensor(out=zt[:], in0=z[:], in1=fr[:],
                                    op=ALU.mult)
            tval = st([1, 1], F32, "tval")
            nc.vector.tensor_reduce(out=tval[:], in_=zt[:],
                                    axis=mybir.AxisListType.X, op=ALU.add)
            t_bc = st([128, 1], F32, "t_bc")
            nc.gpsimd.partition_broadcast(t_bc[:], tval[:])

            if debug_outputs:
                nc.sync.dma_start(dbg_g1.ap(), stage1["g1"][:])
                nc.sync.dma_start(dbg_t.ap(), tval[:])
                nc.sync.dma_start(dbg_cnt.ap(), g2[:])
                nc.sync.dma_start(dbg_fin.ap()[:, 0:1], rm1[:])
                nc.sync.dma_start(dbg_fin.ap()[:, 1:2], chi[0:1, :])
                nc.sync.dma_start(dbg_fin.ap()[:, 2:3], tlo[0:1, :])
                nc.sync.dma_start(dbg_fin.ap()[:, 3:4], thi[0:1, :])
                nc.sync.dma_start(dbg_fin.ap()[:, 4:5], C_a[:])
                nc.sync.dma_start(dbg_fin.ap()[:, 5:6], wa[0:1, :])
                nc.sync.dma_start(dbg_fin.ap()[:, 6:7], tau_a[:])
                nc.sync.dma_start(dbg_fin.ap()[:, 7:8], tau_b[:])
                nc.sync.dma_start(dbg_cand.ap(), cand[:])

            # ============ Phase 5: mask + decode + reduce, chunked ======
            if not host_reduce:
                partial = drp.tile([B, D], BF16, tag="partial",
                                   name="partial")
            for bh in range(NBH):
                # mask this chunk's batch columns for all feature tiles
                ft_t = []
                for fc in range(FT):
                    if bh == 0 and fc < 3:
                        pr = pr_pre[fc]
                    else:
                        pr = st([128, RB], F32, "rld", bufs=3)
                        nc.sync.dma_start(
                            pr[:], postT_dram[fc * 128:(fc + 1) * 128,
                                              bh * RB:(bh + 1) * RB])
                    ftt = st([128, RB], BF16, "ft", bufs=FT + 8)
                    nc.vector.scalar_tensor_tensor(
                        out=ftt[:], in0=pr[:], scalar=t_bc[:], in1=pr[:],
                        op0=ALU.is_ge, op1=ALU.mult)
                    ft_t.append(ftt)
                for dhp in range(D // HB):
                    pst = [psp.tile([128, HB], F32, tag="ps", bufs=4,
                                    name=f"psd{b}") for b in range(BPC)]
                    for fc in range(FT):
                        wdt = wd_t[fc][dhp]
                        for b in range(BPC):
                            for c in range(0, HB, 512):
                                nc.tensor.matmul(
                                    pst[b][:, c:c + 512],
                                    ft_t[fc][:, b * 128:(b + 1) * 128],
                                    wdt[:, c:c + 512],
                                    start=(fc == 0), stop=(fc == FT - 1))
                    for b in range(BPC):
                        xe = st([128, HB], BF16, "ev2", bufs=4)
                        nc.scalar.activation(xe[:], pst[b][:], ACTF.Copy)
                        row0 = bh * RB + b * 128
                        if host_reduce:
                            nc.sync.dma_start(
                                out_d.ap()[row0:row0 + 128,
                                           dhp * HB:(dhp + 1) * HB], xe[:])
                        else:
                            nc.sync.dma_start(
                                partial[row0:row0 + 128,
                                        dhp * HB:(dhp + 1) * HB], xe[:])
                if not host_reduce:
                    rs_out = drp.tile([SH, D], BF16, tag=f"rs_out{bh}",
                                      name=f"rs_out{bh}")
                    nc.gpsimd.collective_compute(
                        "ReduceScatter", ALU.add,
                        ins=[partial[bh * RB:(bh + 1) * RB, :]],
                        outs=[rs_out.opt()],
                        replica_groups=rg)
                    nc.sync.dma_start(
                        out_d.ap()[bh * SH:(bh + 1) * SH, :], rs_out[:])

    nc.compile()
    return nc


@functools.lru_cache(maxsize=2)
def _get_program(B, D, F, K_total, debug_outputs=False, host_reduce=False):
    return build(B, D, F, K_total, debug_outputs, host_reduce)


def _split_fp16(a):
    """a -> (fp16(a), residual fp32)."""
    hi = a.astype(np.float16)
    lo = a - hi.astype(np.float32)
    return hi, lo


def make_inputs(x, W_enc, b_enc, W_dec, b_dec, k):
    B, D = x.shape
    F = W_enc.shape[0]
    FC = F // N_CORES
    FT = FC // 128
    DT = D // 128
    NH = 2
    HB = B // NH
    E4 = ml_dtypes.float8_e4m3
    xT = np.ascontiguousarray((np.asarray(x, np.float32)
                               - np.asarray(b_dec, np.float32)[None, :]).T)
    x16, xl = _split_fp16(xT)
    x16s = np.ascontiguousarray(
        (x16.astype(np.float32) * 128.0).astype(np.float16))
    xp = np.empty((D, NH, 2, HB), dtype=E4)
    for h in range(NH):
        xp[:, h, 0, :] = (xl[:, h * HB:(h + 1) * HB] * 4096.0).astype(E4)
        xp[:, h, 1, :] = (x16[:, h * HB:(h + 1) * HB].astype(np.float32)
                          * 4.0).astype(E4)
    xp = np.ascontiguousarray(xp.reshape(D, 2 * B))
    pr1 = _ladder().reshape(128, 1)
    prrow = _ladder().reshape(1, 128)
    j2 = np.linspace(0.0, 1.0, NP2, dtype=np.float32).reshape(1, NP2)
    j128 = (np.arange(128, dtype=np.float32) / 128.0).reshape(128, 1)
    j16 = np.arange(16, dtype=np.float32).reshape(1, 16)
    in_maps = []
    for c in range(N_CORES):
        weT = np.ascontiguousarray(
            np.asarray(W_enc, np.float32)[c * FC:(c + 1) * FC, :].T)
        w16, wl = _split_fp16(weT)
        # pre-tiled fp16 hi: [128, FT*DT*128], contiguous per fc slice
        weh = np.ascontiguousarray(
            (w16.astype(np.float32) * 128.0).astype(np.float16)
            .reshape(DT, 128, FT, 128).transpose(1, 2, 0, 3)
            .reshape(128, FT * DT * 128))
        # fp8 pairs: [128, FT*DT*2*128] = (w16*4, wl*4096) per d-tile
        wep8 = np.empty((DT, 128, FT, 2, 128), dtype=E4)
        w16r = (w16.astype(np.float32) * 4.0).reshape(DT, 128, FT, 128)
        wlr = (wl * 4096.0).reshape(DT, 128, FT, 128)
        wep8[:, :, :, 0, :] = w16r.astype(E4)
        wep8[:, :, :, 1, :] = wlr.astype(E4)
        wep = np.ascontiguousarray(
            wep8.transpose(1, 2, 0, 3, 4).reshape(128, FT * DT * 256))
        wdT = np.ascontiguousarray(
            np.asarray(W_dec, np.float32)[:, c * FC:(c + 1) * FC].T)
        wd = wdT.astype(ml_dtypes.bfloat16)
        be = np.ascontiguousarray(
            np.asarray(b_enc, np.float32)[c * FC:(c + 1) * FC]
            .reshape(FT, 128).T)
        in_maps.append({
            "x16": x16s, "xp": xp, "weh": weh, "wep": wep, "wd": wd,
            "be": be, "pr1": pr1, "prrow": prrow, "j2": j2,
            "j128": j128, "j16": j16,
        })
    return in_maps


def kernel(x, W_enc, b_enc, W_dec, b_dec, k, _debug=False,
           _host_reduce=True, _trace=False):
    x = np.asarray(x)
    B, D = x.shape
    F = np.asarray(W_enc).shape[0]
    K_total = int(k) * B
    nc = _get_program(B, D, F, K_total, _debug, _host_reduce)
    in_maps = make_inputs(x, W_enc, b_enc, W_dec, b_dec, k)
    res = bass_utils.run_bass_kernel_spmd(
        nc, in_maps, core_ids=list(range(N_CORES)), trace=_trace)
    b_dec32 = np.asarray(b_dec, np.float32)
    if _host_reduce:
        acc = np.zeros((B, D), dtype=np.float32)
        for c in range(N_CORES):
            acc += np.asarray(res.results[c]["out"], dtype=np.float32)
        out = acc + b_dec32[None, :]
    else:
        NBH = 4
        SH = B // NBH // N_CORES
        out = np.empty((B, D), dtype=np.float32)
        for r in range(N_CORES):
            sh = np.asarray(res.results[r]["out"],
                            dtype=np.float32).reshape(NBH, SH, D)
            for c in range(NBH):
                out[c * (B // NBH) + r * SH:
                    c * (B // NBH) + (r + 1) * SH] = sh[c]
        out = out + b_dec32[None, :]
    if _debug or _trace:
        kernel.last_results = res
    return out.astype(np.float32)
